# revision 32
# baseline (speedup 1.0000x reference)
"""BiMamba layer (fwd+bwd selective-scan mamba blocks + FFN) on 8 Trainium2
NeuronCores via Bass/Tile.

Sharding: data-parallel over batch - core i processes sample i (B=8).
Layout: channel-major [channel_partitions, time] on device; host pre-transposes
x and weights, output is returned transposed and the host transposes back.

v2 design vs baseline:
- bf16 everywhere (matmul weights + streams): DVE tensor_tensor at 2x,
  tensor_scalar at 4x, halved DMA/SBUF footprint.
- depthwise causal conv FUSED into in_proj: host ships 4 time-shift weight
  matrices W_j = in_w_xi * conv_w[:, j]; device accumulates 16 (k x j)
  matmuls per d-block into PSUM over a haloed x tile. No DVE conv, no halo
  bookkeeping tiles.
- decay path stays in ONE ACT table (exp/ln): E = exp(u + dt_b),
  dt = ln(E + 1) [= softplus], w = exp(-dt); dA powers w^2..w^16 via ACT
  squares + 3 batched bf16 DVE muls. (The old sigmoid/ln mix forced ~16
  1.28us ACT table swaps per chunk on HW.)
- residual adds via identity matmuls accumulated into the out_proj / FFN
  PSUM, LN inputs come from single ACT copies; FFN phase has near-zero DVE.
- B/C row-broadcasts: 4-state batched PSUM + one ACT copy per group.

The sequential selective scan uses the DVE tensor_tensor_scan instruction
chunked over time with running state carried between chunks via breaker
columns (decay 0). Backward direction runs in natural time order with
anti-causal conv windows and right-to-left scans via negative strides.
"""

import sys

sys.path.insert(0, "/opt/trn_rl_repo")

import numpy as np

import concourse.bass as bass
import concourse.mybir as mybir
import concourse.tile as tile

F32 = mybir.dt.float32
BF16 = mybir.dt.bfloat16
AF = mybir.ActivationFunctionType
ALU = mybir.AluOpType

D_MODEL = 512
D_FF = 2048
D_STATE = 16
D_CONV = 4
D_INNER = 1024
DT_RANK = 32
EPS = 1e-5

N_CORES = 8
L_FULL = 4096
T_CHUNK = 256

# ----------------------------------------------------------------------------
# walrus workaround: this compiler build rejects >1 semaphore wait per
# instruction. Hoist excess waits onto same-engine NoOps placed just before
# the instruction (engines execute their queue in order, so semantics hold).
# ----------------------------------------------------------------------------
_wait_ctr = [0]


def split_multi_waits(nc, max_waits=1):
    for f in nc.m.functions:
        for blk in f.blocks:
            insts = list(blk.instructions)
            out = []
            changed = False
            for inst in insts:
                si = inst.sync_info
                waits = list(si.on_wait) if si and si.on_wait else []
                if len(waits) > max_waits:
                    changed = True
                    extra, keep = waits[:-max_waits], waits[-max_waits:]
                    for w in extra:
                        _wait_ctr[0] += 1
                        nop = mybir.InstNoOp(name=f"I-waitsplit-{_wait_ctr[0]}")
                        nop.engine = inst.engine
                        nop.sync_info = mybir.SyncInfo(on_wait=[w], on_update=[])
                        out.append(nop)
                    si.on_wait = keep
                out.append(inst)
            if changed:
                blk.instructions = out


# ----------------------------------------------------------------------------
# device program builder
# ----------------------------------------------------------------------------
def build_program(L=L_FULL, T=T_CHUNK, n_cores=N_CORES, repeat=1, **_ignored):
    C = L // T
    assert C * T == L
    ND = D_INNER // 128   # 8 d-blocks
    NM = D_MODEL // 128   # 4 k-tiles of d_model
    NF = D_FF // 128      # 16 m-tiles of d_ff

    nc = bass.Bass("TRN2", target_bir_lowering=False, debug=False,
                   num_devices=n_cores)

    def par(name, shape, out=False, dt=F32):
        return nc.declare_dram_parameter(name, list(shape), dt, isOutput=out)

    xT = par("xT", (D_MODEL, L), dt=BF16)
    outT = par("outT", (D_MODEL, L), out=True)
    W = {}
    for p in ("f", "b"):
        W[p] = dict(
            z_wT=par(f"{p}_z_wT", (D_MODEL, D_INNER), dt=BF16),
            cw0T=par(f"{p}_cw0T", (D_MODEL, D_INNER), dt=BF16),
            cw1T=par(f"{p}_cw1T", (D_MODEL, D_INNER), dt=BF16),
            cw2T=par(f"{p}_cw2T", (D_MODEL, D_INNER), dt=BF16),
            cw3T=par(f"{p}_cw3T", (D_MODEL, D_INNER), dt=BF16),
            out_wT=par(f"{p}_out_wT", (D_INNER, D_MODEL), dt=BF16),
            xp_wT=par(f"{p}_xp_wT", (D_INNER, DT_RANK + 2 * D_STATE), dt=BF16),
            dt_wT=par(f"{p}_dt_wT", (DT_RANK, D_INNER), dt=BF16),
            conv_bT=par(f"{p}_conv_bT", (2, D_INNER), dt=BF16),
            dt_bT=par(f"{p}_dt_bT", (2, D_INNER), dt=BF16),
            D=par(f"{p}_D", (D_INNER, 1)),
        )
    LN = {k: par(k, (D_MODEL, 1)) for k in
          ("lnf_g", "lnf_b", "lnb_g", "lnb_b", "lnff_g", "lnff_b")}
    w1T = par("w1T", (D_MODEL, D_FF), dt=BF16)
    b1 = par("b1", (D_FF, 1))
    w2T = par("w2T", (D_FF, D_MODEL), dt=BF16)
    b2 = par("b2", (D_MODEL, 1))
    selbc = par("selbc", (48, 16 * 128), dt=BF16)
    ident = par("ident", (128, 128), dt=BF16)

    of_d = nc.dram_tensor("of_d", [D_MODEL, L], BF16)
    ob_d = nc.dram_tensor("ob_d", [D_MODEL, L], BF16)

    with tile.TileContext(nc) as tc:
        with tc.tile_pool(name="const", bufs=1) as cpool:
            ones_col = cpool.tile([128, 1], BF16, tag="ones_col", name="ones_col")
            nc.vector.memset(ones_col[:], 1.0)
            ones_row = cpool.tile([1, 128], BF16, tag="ones_row", name="ones_row")
            nc.vector.memset(ones_row[:], 1.0)
            eps_t = cpool.tile([1, 1], F32, tag="eps_t", name="eps_t")
            nc.vector.memset(eps_t[:], EPS)
            ones_rT = cpool.tile([2, T], BF16, tag="ones_rT", name="ones_rT")
            nc.vector.memset(ones_rT[:], 1.0)
            # Row-broadcast selection matmuls (B from dbc rows 32..47 base 32,
            # C from csb base 0); pattern shipped from host.
            ones_bc = cpool.tile([48, 16 * 128], BF16, tag="ones_bc",
                                 name="ones_bc")
            nc.sync.dma_start(ones_bc[:], selbc[:])
            id128 = cpool.tile([128, 128], BF16, tag="id128", name="id128")
            nc.sync.dma_start(id128[:], ident[:])

            for _rep in range(repeat):
                _direction(nc, tc, W["f"], LN, xT, of_d, True, "f",
                           L, T, C, ND, NM, ones_col, ones_row, ones_bc,
                           id128, eps_t, ones_rT, None)
                ffn = dict(w1T=w1T, b1=b1, w2T=w2T, b2=b2, of_d=of_d,
                           outT=outT)
                _direction(nc, tc, W["b"], LN, xT, None, False, "b",
                           L, T, C, ND, NM, ones_col, ones_row, ones_bc,
                           id128, eps_t, ones_rT, ffn)

    return nc


def _load_weights(nc, wp, w, lng_name, lnb_name, LN, ND, NM):
    """DMA per-direction weights into persistent sbuf tiles (all bf16),
    ordered by first use (cw/zw feed the first in_proj; outw/LN come a
    chunk later)."""
    s = {}
    s["cw"] = []
    for j in range(D_CONV):
        tiles = [wp.tile([128, D_INNER], BF16, tag=f"cw{j}_{k}",
                         name=f"cw{j}_{k}") for k in range(NM)]
        for k in range(NM):
            nc.sync.dma_start(tiles[k][:],
                              w[f"cw{j}T"][128 * k:128 * (k + 1), :])
        s["cw"].append(tiles)
    s["cbT"] = wp.tile([2, D_INNER], BF16, tag="cbT", name="cbT")
    nc.sync.dma_start(s["cbT"][:], w["conv_bT"][:])
    s["dbT"] = wp.tile([2, D_INNER], BF16, tag="dbT", name="dbT")
    nc.sync.dma_start(s["dbT"][:], w["dt_bT"][:])
    s["zw"] = [wp.tile([128, D_INNER], BF16, tag=f"zw{k}", name=f"zw{k}")
               for k in range(NM)]
    for k in range(NM):
        nc.sync.dma_start(s["zw"][k][:], w["z_wT"][128 * k:128 * (k + 1), :])
    s["xpw"] = [wp.tile([128, DT_RANK + 2 * D_STATE], BF16, tag=f"xpw{k}",
                        name=f"xpw{k}") for k in range(ND)]
    for k in range(ND):
        nc.sync.dma_start(s["xpw"][k][:], w["xp_wT"][128 * k:128 * (k + 1), :])
    s["dtw"] = wp.tile([DT_RANK, D_INNER], BF16, tag="dtw", name="dtw")
    nc.sync.dma_start(s["dtw"][:], w["dt_wT"][:])
    s["Dp"] = [wp.tile([128, 1], F32, tag=f"Dp{d}", name=f"Dp{d}")
               for d in range(ND)]
    for d in range(ND):
        nc.sync.dma_start(s["Dp"][d][:], w["D"][128 * d:128 * (d + 1), :])
    s["outw"] = [wp.tile([128, D_MODEL], BF16, tag=f"outw{k}", name=f"outw{k}")
                 for k in range(ND)]
    for k in range(ND):
        nc.sync.dma_start(s["outw"][k][:], w["out_wT"][128 * k:128 * (k + 1), :])
    s["lng"] = [wp.tile([128, 1], F32, tag=f"lng{k}", name=f"lng{k}")
                for k in range(NM)]
    s["lnb"] = [wp.tile([128, 1], F32, tag=f"lnb{k}", name=f"lnb{k}")
                for k in range(NM)]
    for k in range(NM):
        nc.sync.dma_start(s["lng"][k][:], LN[lng_name][128 * k:128 * (k + 1), :])
        nc.sync.dma_start(s["lnb"][k][:], LN[lnb_name][128 * k:128 * (k + 1), :])
    return s


def _layernorm(nc, ln_in, lng, lnb, psS, psM, smtmp, lnout_pool, ones_col,
               ones_row, eps_t, T, NM, tag, out_dt=BF16):
    """LN over the channel dim (NM k-tiles of 128 partitions).
    ln_in: list of NM bf16 sbuf tiles [128, T]. Returns output tiles."""
    ps_sq = psS.tile([1, 2 * T], F32, tag="stat", name="stat")
    ps_s = ps_sq[:, 0:T]
    ps_q = ps_sq[:, T:2 * T]
    sq = [None] * NM
    for k in range(NM):
        sq[k] = smtmp.tile([128, T], BF16, tag="sq", name="sq", bufs=4)
        nc.scalar.square(sq[k][:], ln_in[k][:])
    for k in range(NM):
        nc.tensor.matmul(ps_s, ones_col[:], ln_in[k][:],
                         start=(k == 0), stop=(k == NM - 1))
    for k in range(NM):
        nc.tensor.matmul(ps_q, ones_col[:], sq[k][:],
                         start=(k == 0), stop=(k == NM - 1))
    mu = smtmp.tile([1, T], BF16, tag="mu", name="mu", bufs=1)
    nc.scalar.activation(mu[:], ps_s, AF.Copy, scale=1.0 / D_MODEL)
    m2 = smtmp.tile([1, T], F32, tag="m2", name="m2", bufs=1)
    nc.scalar.activation(m2[:], ps_q, AF.Copy, scale=1.0 / D_MODEL)
    mu2 = smtmp.tile([1, T], F32, tag="mu2", name="mu2", bufs=1)
    nc.scalar.square(mu2[:], mu[:])
    var = smtmp.tile([1, T], F32, tag="var", name="var", bufs=1)
    nc.vector.tensor_sub(var[:], m2[:], mu2[:])
    # rstd = exp(-0.5*ln(var+eps))  (stays in the Exp/Ln ACT table)
    lnv = smtmp.tile([1, T], F32, tag="lnv", name="lnv", bufs=1)
    nc.scalar.activation(lnv[:], var[:], AF.Ln, bias=eps_t[:])
    rstd = smtmp.tile([1, T], BF16, tag="rstd", name="rstd", bufs=1)
    nc.scalar.activation(rstd[:], lnv[:], AF.Exp, scale=-0.5)
    # broadcast mu/rstd to 128 partitions; ACT-copy to bf16 sbuf
    ps_mr = psM.tile([128, 2 * T], F32, tag="bcst", name="bcst")
    nc.tensor.matmul(ps_mr[:, 0:T], ones_row[:], mu[:], start=True, stop=True)
    nc.tensor.matmul(ps_mr[:, T:2 * T], ones_row[:], rstd[:], start=True,
                     stop=True)
    mrb = smtmp.tile([128, 2 * T], BF16, tag="mrb", name="mrb", bufs=1)
    nc.scalar.copy(mrb[:], ps_mr[:])
    mub = mrb[:, 0:T]
    rsb = mrb[:, T:2 * T]
    outs = []
    for k in range(NM):
        t1 = smtmp.tile([128, T], BF16, tag="lt1", name="lt1", bufs=2)
        nc.vector.tensor_sub(t1[:], ln_in[k][:], mub)
        t2 = smtmp.tile([128, T], BF16, tag="lt2", name="lt2", bufs=2)
        nc.vector.tensor_mul(t2[:], t1[:], rsb)
        o = lnout_pool.tile([128, T], out_dt, tag=tag)
        nc.vector.tensor_scalar(o[:], t2[:], lng[k][:], lnb[k][:],
                                op0=ALU.mult, op1=ALU.add)
        outs.append(o)
    return outs


def _direction(nc, tc, w, LN, xT, stage_d, fwd, p, L, T, C, ND, NM,
               ones_col, ones_row, ones_bc, id128, eps_t, ones_rT, ffn):
    """One mamba direction, software-pipelined:
    per chunk: [shared prep ci] -> [deferred out_proj+LN of ci-1 (+FFN chunk
    when fused)] -> [d-loop ci with one-d-block-deferred reduce tails].
    When ffn is not None (backward direction), the FFN+final-LN for time
    chunk j=C-ci is emitted right after outLN(ci-1), consuming the backward
    LN outputs straight from SBUF (no DRAM staging)."""
    from contextlib import ExitStack
    T1 = T + 1
    doff = 1 if fwd else 0          # data column offset in [.., T1]
    boff = 0 if fwd else T          # breaker column
    zoff = 3 if fwd else 0          # data window offset in haloed xk
    with ExitStack() as ctx:
        wp = ctx.enter_context(tc.tile_pool(name=f"w_{p}", bufs=1))
        sw = _load_weights(nc, wp, w, f"ln{p}_g", f"ln{p}_b", LN, ND, NM)
        if ffn is not None:
            fw = _load_ffn_weights(nc, wp, ffn, LN, NM, D_FF // 128)

        xk_pool = ctx.enter_context(tc.tile_pool(name=f"xk_{p}", bufs=8))
        tmp_pool = ctx.enter_context(tc.tile_pool(name=f"tmp_{p}", bufs=3))
        xc_pool = ctx.enter_context(tc.tile_pool(name=f"xc_{p}", bufs=5))
        zs_pool = ctx.enter_context(tc.tile_pool(name=f"zs_{p}", bufs=5))
        g_pool = ctx.enter_context(tc.tile_pool(name=f"g_{p}", bufs=2))
        dbc_pool = ctx.enter_context(tc.tile_pool(name=f"dbc_{p}", bufs=2))
        repB_pool = ctx.enter_context(tc.tile_pool(name=f"repB_{p}", bufs=1))
        repC_pool = ctx.enter_context(tc.tile_pool(name=f"repC_{p}", bufs=1))
        dA_pool = ctx.enter_context(tc.tile_pool(name=f"dA_{p}", bufs=3))
        b_pool = ctx.enter_context(tc.tile_pool(name=f"b_{p}", bufs=2))
        st_pool = ctx.enter_context(tc.tile_pool(name=f"st_{p}", bufs=2))
        ys_pool = ctx.enter_context(tc.tile_pool(name=f"ys_{p}", bufs=8))
        y_pool = ctx.enter_context(tc.tile_pool(name=f"y_{p}", bufs=2))
        ln_pool = ctx.enter_context(tc.tile_pool(name=f"ln_{p}", bufs=4))
        lo_pool = ctx.enter_context(tc.tile_pool(name=f"lo_{p}", bufs=3))
        if ffn is not None:
            fio_pool = ctx.enter_context(tc.tile_pool(name="ffn_io", bufs=4))
            fh_pool = ctx.enter_context(tc.tile_pool(name="ffn_h", bufs=4))
            fh1_pool = ctx.enter_context(tc.tile_pool(name="ffn_h1", bufs=16))
            fln_pool = ctx.enter_context(tc.tile_pool(name="ffn_ln", bufs=4))
            flo_pool = ctx.enter_context(tc.tile_pool(name="ffn_lo", bufs=3))

        psA = ctx.enter_context(tc.tile_pool(name=f"psA_{p}", bufs=2, space="PSUM"))
        psB = ctx.enter_context(tc.tile_pool(name=f"psB_{p}", bufs=2, space="PSUM"))
        psS = ctx.enter_context(tc.tile_pool(name=f"psS_{p}", bufs=2, space="PSUM"))
        psM = ctx.enter_context(tc.tile_pool(name=f"psM_{p}", bufs=2, space="PSUM"))

        state_prev = [None] * ND
        pend = None   # (t0, xk, ys_tiles) of previous chunk

        def emit_outln(pd):
            pt0, pxk, pys = pd
            ln_in = [None] * NM
            for m in range(NM):
                ps = psA.tile([128, T], F32, tag="mm", name="mm")
                for k in range(ND):
                    nc.tensor.matmul(ps[:],
                                     sw["outw"][k][:, 128 * m:128 * (m + 1)],
                                     pys[k][:], start=(k == 0), stop=False)
                nc.tensor.matmul(ps[:], id128[:], pxk[m][:, zoff:zoff + T],
                                 start=False, stop=True)
                li = ln_pool.tile([128, T], BF16, tag="lnin", name="lnin")
                nc.scalar.copy(li[:], ps[:])
                ln_in[m] = li
            outs = _layernorm(nc, ln_in, sw["lng"], sw["lnb"], psS, psM,
                              tmp_pool, lo_pool, ones_col, ones_row, eps_t,
                              T, NM, tag="lo")
            if stage_d is not None:
                for m in range(NM):
                    nc.sync.dma_start(
                        stage_d[128 * m:128 * (m + 1), pt0:pt0 + T],
                        outs[m][:])
            return pt0, outs

        def emit_ffn_a(ob_tiles, of_tiles):
            NF = D_FF // 128
            hk = [None] * NM
            for k in range(NM):
                a = of_tiles[k]
                ps = psA.tile([128, T], F32, tag="mm", name="mm")
                nc.tensor.matmul(ps[:], id128[:], a[:], start=True, stop=False)
                nc.tensor.matmul(ps[:], id128[:], ob_tiles[k][:],
                                 start=False, stop=True)
                h = fh_pool.tile([128, T], BF16, tag="h", name="h")
                nc.scalar.activation(h[:], ps[:], AF.Copy, scale=0.5)
                hk[k] = h
            h1 = [None] * NF
            for m in range(NF):
                ps = psA.tile([128, T], F32, tag="mm", name="mm")
                for k in range(NM):
                    nc.tensor.matmul(ps[:],
                                     fw["w1"][k][:, 128 * m:128 * (m + 1)],
                                     hk[k][:], start=(k == 0),
                                     stop=(k == NM - 1))
                t = fh1_pool.tile([128, T], BF16, tag="h1", name="h1")
                nc.scalar.activation(t[:], ps[:], AF.Gelu_apprx_tanh,
                                     bias=fw["b1"][m][:])
                h1[m] = t
            return hk, h1

        def emit_ffn_b(t0, mid):
            NF = D_FF // 128
            hk, h1 = mid
            ln_in = [None] * NM
            for m in range(NM):
                ps = psA.tile([128, T], F32, tag="mm", name="mm")
                for k in range(NF):
                    nc.tensor.matmul(ps[:],
                                     fw["w2"][k][:, 128 * m:128 * (m + 1)],
                                     h1[k][:], start=(k == 0), stop=False)
                nc.tensor.matmul(ps[:], id128[:], hk[m][:],
                                 start=False, stop=True)
                li = fln_pool.tile([128, T], BF16, tag="lnin", name="lnin")
                nc.scalar.activation(li[:], ps[:], AF.Identity,
                                     bias=fw["b2"][m][:])
                ln_in[m] = li
            outs = _layernorm(nc, ln_in, fw["lng"], fw["lnb"], psS, psM,
                              tmp_pool, flo_pool, ones_col, ones_row, eps_t,
                              T, NM, tag="lo", out_dt=F32)
            for m in range(NM):
                nc.sync.dma_start(
                    ffn["outT"][128 * m:128 * (m + 1), t0:t0 + T], outs[m][:])

        for ci in range(C):
            j = ci if fwd else (C - 1 - ci)      # time-chunk index
            t0 = j * T

            # ---- shared prep: haloed x loads ----
            xk = []
            for k in range(NM):
                t = xk_pool.tile([128, T + 3], BF16, tag="xk", name="xk")
                r0, r1 = 128 * k, 128 * (k + 1)
                if fwd:
                    if ci == 0:
                        nc.vector.memset(t[:, 0:3], 0.0)
                        nc.sync.dma_start(t[:, 3:3 + T], xT[r0:r1, 0:T])
                    else:
                        nc.sync.dma_start(t[:], xT[r0:r1, t0 - 3:t0 + T])
                else:
                    if ci == 0:
                        nc.sync.dma_start(t[:, 0:T], xT[r0:r1, t0:t0 + T])
                        nc.vector.memset(t[:, T:T + 3], 0.0)
                    else:
                        nc.sync.dma_start(t[:], xT[r0:r1, t0:t0 + T + 3])
                xk.append(t)

            # ---- shared prep: fused in_proj+conv -> xc, z -> zs ----
            # pairs of d-blocks share one [128,2T] psum + ONE silu ACT;
            # conv bias enters via a rank-1 matmul (convbT x ones_row)
            xc_tiles = [None] * ND
            zs_tiles = [None] * ND
            for mp in range(ND // 2):
                ps2 = psB.tile([128, 2, T], F32, tag="bc", name="bc")
                for half in range(2):
                    m = 2 * mp + half
                    for jj in range(D_CONV):
                        for k in range(NM):
                            nc.tensor.matmul(
                                ps2[:, half, :],
                                sw["cw"][jj][k][:, 128 * m:128 * (m + 1)],
                                xk[k][:, jj:jj + T],
                                start=(jj == 0 and k == 0), stop=False)
                    nc.tensor.matmul(ps2[:, half, :],
                                     sw["cbT"][:, 128 * m:128 * (m + 1)],
                                     ones_rT[:], start=False, stop=True)
                xc2 = xc_pool.tile([128, 2, T], BF16, tag="xc", name="xc")
                nc.scalar.activation(xc2[:], ps2[:], AF.Silu)
                xc_tiles[2 * mp] = xc2[:, 0, :]
                xc_tiles[2 * mp + 1] = xc2[:, 1, :]
            for mp in range(ND // 2):
                ps2 = psB.tile([128, 2, T], F32, tag="bc", name="bc")
                for half in range(2):
                    m = 2 * mp + half
                    for k in range(NM):
                        nc.tensor.matmul(ps2[:, half, :],
                                         sw["zw"][k][:, 128 * m:128 * (m + 1)],
                                         xk[k][:, zoff:zoff + T],
                                         start=(k == 0), stop=(k == NM - 1))
                zs2 = zs_pool.tile([128, 2, T], BF16, tag="zs", name="zs")
                nc.scalar.activation(zs2[:], ps2[:], AF.Silu)
                zs_tiles[2 * mp] = zs2[:, 0, :]
                zs_tiles[2 * mp + 1] = zs2[:, 1, :]

            # ---- shared prep: xproj -> dbc [48,T], csb [16,T] ----
            psd = psA.tile([DT_RANK + D_STATE, T], F32, tag="mm", name="mm")
            for k in range(ND):
                nc.tensor.matmul(psd[:], sw["xpw"][k][:, :DT_RANK + D_STATE],
                                 xc_tiles[k][:], start=(k == 0),
                                 stop=(k == ND - 1))
            dbc = dbc_pool.tile([DT_RANK + D_STATE, T], BF16, tag="dbc",
                                name="dbc")
            nc.scalar.copy(dbc[:], psd[:])
            psc = psA.tile([D_STATE, T], F32, tag="mm", name="mm")
            for k in range(ND):
                nc.tensor.matmul(psc[:], sw["xpw"][k][:, DT_RANK + D_STATE:],
                                 xc_tiles[k][:], start=(k == 0),
                                 stop=(k == ND - 1))
            csb = dbc_pool.tile([D_STATE, T], BF16, tag="csb", name="csb")
            nc.scalar.copy(csb[:], psc[:])

            # ---- shared prep: broadcast B,C rows (2-state batches) ----
            Brep = repB_pool.tile([128, D_STATE, T], BF16, tag="brep",
                                 name="brep")
            Crep = repC_pool.tile([128, D_STATE, T], BF16, tag="crep",
                                 name="crep")
            for q in range(8):
                pb = psB.tile([128, 2, T], F32, tag="bc", name="bc")
                for n2 in range(2):
                    n = 2 * q + n2
                    nc.tensor.matmul(pb[:, n2, :],
                                     ones_bc[32:48, 128 * n:128 * (n + 1)],
                                     dbc[DT_RANK:DT_RANK + D_STATE, :],
                                     start=True, stop=True)
                nc.scalar.copy(Brep[:, 2 * q:2 * (q + 1), :], pb[:])
                pc = psB.tile([128, 2, T], F32, tag="bc", name="bc")
                for n2 in range(2):
                    n = 2 * q + n2
                    nc.tensor.matmul(pc[:, n2, :],
                                     ones_bc[0:16, 128 * n:128 * (n + 1)],
                                     csb[:], start=True, stop=True)
                nc.scalar.copy(Crep[:, 2 * q:2 * (q + 1), :], pc[:])

            # ---- prefetch FFN of-tiles for the NEXT slot's time chunk ----
            of_next = None
            if ffn is not None and ci >= 1:
                jf = C - ci            # ffn time-chunk consumed in this slot
                of_next = []
                for k in range(NM):
                    a = fio_pool.tile([128, T], BF16, tag="of", name="of")
                    nc.sync.dma_start(
                        a[:], ffn["of_d"][128 * k:128 * (k + 1),
                                          jf * T:jf * T + T])
                    of_next.append(a)

            # ---- d-loop; deferred out_proj+LN(+FFN) spread over d slots ----
            ys_tiles = [None] * ND
            tails = []
            pt0 = bouts = None
            dt_pair = [None] * (ND // 2)
            for d in range(ND):
                if d == 2 and pend is not None:
                    pt0, bouts = emit_outln(pend)
                if d == 3 and bouts is not None and ffn is not None:
                    ffn_mid = emit_ffn_a(bouts, of_tiles)
                if d == 6 and bouts is not None and ffn is not None:
                    emit_ffn_b(pt0, ffn_mid)
                of_tiles = of_next
                if d % 2 == 0:
                    # paired dt = softplus(u+dt_b) = ln(exp(u+dt_b)+1)
                    dp = d // 2
                    ps2 = psB.tile([128, 2, T], F32, tag="bc", name="bc")
                    for half in range(2):
                        dd = d + half
                        nc.tensor.matmul(
                            ps2[:, half, :],
                            sw["dtw"][:, 128 * dd:128 * (dd + 1)],
                            dbc[0:DT_RANK, :], start=True, stop=False)
                        nc.tensor.matmul(ps2[:, half, :],
                                         sw["dbT"][:, 128 * dd:128 * (dd + 1)],
                                         ones_rT[:], start=False, stop=True)
                    e2 = tmp_pool.tile([128, 2, T], BF16, tag="e", name="e",
                                       bufs=2)
                    nc.scalar.activation(e2[:], ps2[:], AF.Exp)
                    dt2 = tmp_pool.tile([128, 2, T], BF16, tag="dt",
                                        name="dt", bufs=2)
                    nc.scalar.activation(dt2[:], e2[:], AF.Ln, bias=1.0)
                    dt_pair[dp] = dt2
                dtt = dt_pair[d // 2][:, d % 2, :]
                g_t = g_pool.tile([128, T], BF16, tag="g", name="g")
                nc.vector.tensor_mul(g_t[:], dtt, xc_tiles[d][:])

                dA = dA_pool.tile([128, D_STATE, T1], BF16, tag="dA",
                                  name="dA")

                def dpl(i):
                    return dA[:, i, doff:doff + T]

                # powers w^1..w^16 in 1 ACT + 4 doubling DVE muls:
                # p1=w*w; [p2,p3]=[p0,p1]*p1; [p4..7]=[p0..3]*p3;
                # [p8..15]=[p0..7]*p7
                nc.scalar.activation(dpl(0), dtt, AF.Exp, scale=-1.0)
                nc.vector.tensor_mul(dpl(1), dpl(0), dpl(0))
                b2 = dA[:, 1, doff:doff + T].unsqueeze(1).broadcast_to(
                    [128, 2, T])
                nc.vector.tensor_mul(dA[:, 2:4, doff:doff + T], b2,
                                     dA[:, 0:2, doff:doff + T])
                b4 = dA[:, 3, doff:doff + T].unsqueeze(1).broadcast_to(
                    [128, 4, T])
                nc.vector.tensor_mul(dA[:, 4:8, doff:doff + T], b4,
                                     dA[:, 0:4, doff:doff + T])
                b8 = dA[:, 7, doff:doff + T].unsqueeze(1).broadcast_to(
                    [128, 8, T])
                nc.vector.tensor_mul(dA[:, 8:16, doff:doff + T], b8,
                                     dA[:, 0:8, doff:doff + T])
                nc.vector.memset(dA[:, :, boff:boff + 1], 0.0)

                # b = g * B; breaker col = carried state
                bt = b_pool.tile([128, D_STATE, T1], BF16, tag="b", name="b")
                gb = g_t[:].unsqueeze(1).broadcast_to([128, D_STATE, T])
                nc.vector.tensor_mul(bt[:, :, doff:doff + T], gb,
                                     Brep[:, :, :])
                if ci == 0:
                    nc.vector.memset(bt[:, :, boff:boff + 1], 0.0)
                else:
                    nc.vector.tensor_copy(bt[:, :, boff:boff + 1],
                                          state_prev[d][:].unsqueeze(2))
                flat_a = dA[:, :, :].rearrange("p n t -> p (n t)")
                flat_b = bt[:, :, :].rearrange("p n t -> p (n t)")
                if fwd:
                    nc.vector.tensor_tensor_scan(
                        flat_b, flat_a, flat_b, 0.0,
                        op0=ALU.mult, op1=ALU.add)
                else:
                    nc.vector.tensor_tensor_scan(
                        flat_b[:, ::-1], flat_a[:, ::-1], flat_b[:, ::-1],
                        0.0, op0=ALU.mult, op1=ALU.add)
                stt = st_pool.tile([128, D_STATE], BF16, tag=f"st{d}",
                                   name=f"st{d}")
                nc.vector.tensor_copy(stt[:], bt[:, :, T if fwd else 0])
                state_prev[d] = stt
                # yterm = h*C written into the dead dA buffer; all-DVE
                # bf16 fold tree (engines contend for SBUF bandwidth on HW,
                # so gpsimd offload is a net loss); tail deferred two blocks
                yt = dA[:, :, doff:doff + T]
                nc.vector.tensor_mul(yt, bt[:, :, doff:doff + T],
                                     Crep[:, :, :])
                nc.vector.tensor_add(yt[:, 0:8, :], yt[:, 0:8, :],
                                     yt[:, 8:16, :])

                if len(tails) >= 2:
                    tails.pop(0)()

                def make_tail(d=d, t8=yt):
                    def run():
                        nc.vector.tensor_add(t8[:, 0:4, :], t8[:, 0:4, :],
                                             t8[:, 4:8, :])
                        nc.vector.tensor_add(t8[:, 0:2, :], t8[:, 0:2, :],
                                             t8[:, 2:4, :])
                        y_t = y_pool.tile([128, T], BF16, tag="y", name="y")
                        nc.vector.tensor_add(y_t[:], t8[:, 0, :], t8[:, 1, :])
                        yg = y_pool.tile([128, T], BF16, tag="yg", name="yg")
                        nc.vector.tensor_scalar_mul(yg[:], xc_tiles[d][:],
                                                    sw["Dp"][d][:])
                        nc.vector.tensor_add(yg[:], yg[:], y_t[:])
                        ys = ys_pool.tile([128, T], BF16, tag="ys", name="ys")
                        nc.vector.tensor_mul(ys[:], yg[:], zs_tiles[d][:])
                        ys_tiles[d] = ys
                    return run

                tails.append(make_tail())
            for tl in tails:
                tl()
            tails = []
            pend = (t0, xk, ys_tiles)

        if ffn is not None:
            of_last = []
            for k in range(NM):
                a = fio_pool.tile([128, T], BF16, tag="of", name="of")
                nc.sync.dma_start(a[:], ffn["of_d"][128 * k:128 * (k + 1),
                                                    0:T])
                of_last.append(a)
        pt0, bouts = emit_outln(pend)
        if ffn is not None:
            mid = emit_ffn_a(bouts, of_last)
            emit_ffn_b(pt0, mid)


def _load_ffn_weights(nc, wp, ffn, LN, NM, NF):
    fw = {}
    fw["w1"] = [wp.tile([128, D_FF], BF16, tag=f"w1_{k}", name=f"w1_{k}")
                for k in range(NM)]
    for k in range(NM):
        nc.sync.dma_start(fw["w1"][k][:], ffn["w1T"][128 * k:128 * (k + 1), :])
    fw["w2"] = [wp.tile([128, D_MODEL], BF16, tag=f"w2_{k}", name=f"w2_{k}")
                for k in range(NF)]
    for k in range(NF):
        nc.sync.dma_start(fw["w2"][k][:], ffn["w2T"][128 * k:128 * (k + 1), :])
    fw["b1"] = [wp.tile([128, 1], F32, tag=f"fb1_{m}", name=f"fb1_{m}")
                for m in range(NF)]
    for m in range(NF):
        nc.sync.dma_start(fw["b1"][m][:], ffn["b1"][128 * m:128 * (m + 1), :])
    fw["b2"] = [wp.tile([128, 1], F32, tag=f"fb2_{m}", name=f"fb2_{m}")
                for m in range(NM)]
    for m in range(NM):
        nc.sync.dma_start(fw["b2"][m][:], ffn["b2"][128 * m:128 * (m + 1), :])
    fw["lng"] = [wp.tile([128, 1], F32, tag=f"flng{k}", name=f"flng{k}")
                 for k in range(NM)]
    fw["lnb"] = [wp.tile([128, 1], F32, tag=f"flnb{k}", name=f"flnb{k}")
                 for k in range(NM)]
    for k in range(NM):
        nc.sync.dma_start(fw["lng"][k][:],
                          LN["lnff_g"][128 * k:128 * (k + 1), :])
        nc.sync.dma_start(fw["lnb"][k][:],
                          LN["lnff_b"][128 * k:128 * (k + 1), :])
    return fw


# ----------------------------------------------------------------------------
# host side: input packing, cached jitted runner
# ----------------------------------------------------------------------------
def pack_inputs(inputs, n_cores=N_CORES):
    """Shared weight map + per-core input maps. Host-side layout prep:
    transposes, bf16 casts, conv folded into 4 shifted in_proj matrices."""
    f32 = np.float32
    try:
        import ml_dtypes
        bf16 = ml_dtypes.bfloat16
    except ImportError:
        import jax.numpy as jnp
        bf16 = jnp.bfloat16

    def t(a, dt=None):
        return np.ascontiguousarray(
            np.asarray(a, f32).T.astype(dt if dt else f32))

    shared = {}
    for p in ("f", "b"):
        in_w = np.asarray(inputs[f"{p}_in_w"], f32)       # (2048, 512)
        conv_w = np.asarray(inputs[f"{p}_conv_w"], f32)   # (1024, 4)
        xi_w = in_w[:D_INNER]                             # (1024, 512)
        z_w = in_w[D_INNER:]
        shared[f"{p}_z_wT"] = t(z_w, bf16)
        for o in range(D_CONV):
            # device tap offset o reads x[t0-3+o] (fwd) / x[t0+o] i.e.
            # x[t+(3-s)] (bwd); fwd: weight s=o; bwd: weight s=3-o.
            s = o if p == "f" else 3 - o
            Wj = xi_w * conv_w[:, s:s + 1]                # (1024, 512)
            shared[f"{p}_cw{o}T"] = t(Wj, bf16)
        shared[f"{p}_out_wT"] = t(inputs[f"{p}_out_w"], bf16)
        shared[f"{p}_xp_wT"] = t(inputs[f"{p}_xproj_w"], bf16)
        shared[f"{p}_dt_wT"] = t(inputs[f"{p}_dt_w"], bf16)
        def _hilo(v):
            v = np.asarray(v, f32).reshape(1, -1)
            hi = v.astype(bf16)
            lo = (v - hi.astype(f32)).astype(bf16)
            return np.concatenate([hi, lo], axis=0)

        shared[f"{p}_conv_bT"] = _hilo(inputs[f"{p}_conv_b"])
        shared[f"{p}_dt_bT"] = _hilo(inputs[f"{p}_dt_b"])
        shared[f"{p}_D"] = np.asarray(inputs[f"{p}_D"], f32).reshape(-1, 1)
    for src, dst in (("ln_f_g", "lnf_g"), ("ln_f_b", "lnf_b"),
                     ("ln_b_g", "lnb_g"), ("ln_b_b", "lnb_b"),
                     ("ln_ff_g", "lnff_g"), ("ln_ff_b", "lnff_b")):
        shared[dst] = np.asarray(inputs[src], f32).reshape(-1, 1)
    shared["w1T"] = t(inputs["ffn_w1"], bf16)
    shared["b1"] = np.asarray(inputs["ffn_b1"], f32).reshape(-1, 1)
    shared["w2T"] = t(inputs["ffn_w2"], bf16)
    shared["b2"] = np.asarray(inputs["ffn_b2"], f32).reshape(-1, 1)
    sel = np.zeros((48, 16 * 128), f32)
    for k in range(D_STATE):
        sel[k, 128 * k:128 * (k + 1)] = 1.0
        sel[32 + k, 128 * k:128 * (k + 1)] = 1.0
    shared["selbc"] = sel.astype(bf16)
    shared["ident"] = np.eye(128, dtype=f32).astype(bf16)

    x = np.asarray(inputs["x"], f32)
    in_maps = []
    for i in range(n_cores):
        m = dict(shared)
        m["xT"] = np.ascontiguousarray(x[i].T.astype(bf16))
        in_maps.append(m)
    return in_maps


_RUNNER = {}


def make_runner(**build_kwargs):
    import jax
    import jax.numpy as jnp
    from jax.experimental.shard_map import shard_map
    from jax.sharding import Mesh, NamedSharding, PartitionSpec
    from concourse import bass2jax

    nc = build_program(**build_kwargs)
    split_multi_waits(nc)
    bass2jax.install_neuronx_cc_hook()

    partition_name = (nc.partition_id_tensor.name
                      if nc.partition_id_tensor else None)
    in_names, out_names, out_avals, zero_shapes = [], [], [], []
    for alloc in nc.m.functions[0].allocations:
        if not isinstance(alloc, mybir.MemoryLocationSet):
            continue
        name = alloc.memorylocations[0].name
        if alloc.kind == "ExternalInput":
            if name != partition_name:
                in_names.append(name)
        elif alloc.kind == "ExternalOutput":
            shape = tuple(alloc.tensor_shape)
            dtype = mybir.dt.np(alloc.dtype)
            out_names.append(name)
            out_avals.append(jax.core.ShapedArray(shape, dtype))
            zero_shapes.append((shape, dtype))
    n_params = len(in_names)
    all_in_names = list(in_names) + list(out_names)
    if partition_name is not None:
        all_in_names.append(partition_name)

    def _body(*args):
        operands = list(args)
        if partition_name is not None:
            operands.append(bass2jax.partition_id_tensor())
        outs = bass2jax._bass_exec_p.bind(
            *operands,
            out_avals=tuple(out_avals),
            in_names=tuple(all_in_names),
            out_names=tuple(out_names),
            lowering_input_output_aliases=(),
            sim_require_finite=True,
            sim_require_nnan=True,
            nc=nc,
        )
        return tuple(outs)

    devices = jax.devices()[:N_CORES]
    mesh = Mesh(np.asarray(devices), ("core",))
    n_outs = len(out_avals)
    in_specs = (PartitionSpec("core"),) * (n_params + n_outs)
    out_specs = (PartitionSpec("core"),) * n_outs
    donate = tuple(range(n_params, n_params + n_outs))
    sharded = jax.jit(
        shard_map(_body, mesh=mesh, in_specs=in_specs, out_specs=out_specs,
                  check_rep=False),
        donate_argnums=donate, keep_unused=True)

    sh = NamedSharding(mesh, PartitionSpec("core"))

    def make_zeros():
        return tuple(
            jnp.zeros((N_CORES * s[0],) + tuple(s[1:]), d)
            for s, d in zero_shapes)

    zeros_fn = jax.jit(make_zeros, out_shardings=(sh,) * n_outs)

    return dict(
        fn=sharded, in_names=in_names, out_names=out_names,
        out_avals=out_avals, zeros_fn=zeros_fn, mesh=mesh, sh=sh, jnp=jnp,
        jax=jax)


BEST_CONFIG = dict()


def _get_runner():
    if not _RUNNER:
        _RUNNER.update(make_runner(**BEST_CONFIG))
    return _RUNNER


def _device_inputs(in_maps, r=None):
    import jax
    r = r or _get_runner()
    concat = [np.concatenate([in_maps[c][n] for c in range(N_CORES)], axis=0)
              for n in r["in_names"]]
    return [jax.device_put(a, r["sh"]) for a in concat]


def _run_once(dev_in, r=None):
    r = r or _get_runner()
    zeros = r["zeros_fn"]()
    outs = r["fn"](*dev_in, *zeros)
    return outs


def kernel(**inputs):
    r = _get_runner()
    in_maps = pack_inputs(inputs)
    dev_in = _device_inputs(in_maps)
    outs = _run_once(dev_in)
    outT = np.asarray(outs[r["out_names"].index("outT")])
    outT = outT.reshape(N_CORES, D_MODEL, L_FULL)
    out = np.ascontiguousarray(np.transpose(outT, (0, 2, 1)).astype(np.float32))
    return out


# revision 33
# speedup vs baseline: 1.0409x; 1.0409x over previous
"""BiMamba layer (fwd+bwd selective-scan mamba blocks + FFN) on 8 Trainium2
NeuronCores via Bass/Tile.

Sharding: data-parallel over batch - core i processes sample i (B=8).
Layout: channel-major [channel_partitions, time] on device; host pre-transposes
x and weights, output is returned transposed and the host transposes back.

v2 design vs baseline:
- bf16 everywhere (matmul weights + streams): DVE tensor_tensor at 2x,
  tensor_scalar at 4x, halved DMA/SBUF footprint.
- depthwise causal conv FUSED into in_proj: host ships 4 time-shift weight
  matrices W_j = in_w_xi * conv_w[:, j]; device accumulates 16 (k x j)
  matmuls per d-block into PSUM over a haloed x tile. No DVE conv, no halo
  bookkeeping tiles.
- decay path stays in ONE ACT table (exp/ln): E = exp(u + dt_b),
  dt = ln(E + 1) [= softplus], w = exp(-dt); dA powers w^2..w^16 via ACT
  squares + 3 batched bf16 DVE muls. (The old sigmoid/ln mix forced ~16
  1.28us ACT table swaps per chunk on HW.)
- residual adds via identity matmuls accumulated into the out_proj / FFN
  PSUM, LN inputs come from single ACT copies; FFN phase has near-zero DVE.
- B/C row-broadcasts: 4-state batched PSUM + one ACT copy per group.

The sequential selective scan uses the DVE tensor_tensor_scan instruction
chunked over time with running state carried between chunks via breaker
columns (decay 0). Backward direction runs in natural time order with
anti-causal conv windows and right-to-left scans via negative strides.
"""

import sys

sys.path.insert(0, "/opt/trn_rl_repo")

import numpy as np

import concourse.bass as bass
import concourse.mybir as mybir
import concourse.tile as tile

F32 = mybir.dt.float32
BF16 = mybir.dt.bfloat16
AF = mybir.ActivationFunctionType
ALU = mybir.AluOpType

D_MODEL = 512
D_FF = 2048
D_STATE = 16
D_CONV = 4
D_INNER = 1024
DT_RANK = 32
EPS = 1e-5

N_CORES = 8
L_FULL = 4096
T_CHUNK = 256

# ----------------------------------------------------------------------------
# walrus workaround: this compiler build rejects >1 semaphore wait per
# instruction. Hoist excess waits onto same-engine NoOps placed just before
# the instruction (engines execute their queue in order, so semantics hold).
# ----------------------------------------------------------------------------
_wait_ctr = [0]


def split_multi_waits(nc, max_waits=1):
    for f in nc.m.functions:
        for blk in f.blocks:
            insts = list(blk.instructions)
            out = []
            changed = False
            for inst in insts:
                si = inst.sync_info
                waits = list(si.on_wait) if si and si.on_wait else []
                if len(waits) > max_waits:
                    changed = True
                    extra, keep = waits[:-max_waits], waits[-max_waits:]
                    for w in extra:
                        _wait_ctr[0] += 1
                        nop = mybir.InstNoOp(name=f"I-waitsplit-{_wait_ctr[0]}")
                        nop.engine = inst.engine
                        nop.sync_info = mybir.SyncInfo(on_wait=[w], on_update=[])
                        out.append(nop)
                    si.on_wait = keep
                out.append(inst)
            if changed:
                blk.instructions = out


# ----------------------------------------------------------------------------
# device program builder
# ----------------------------------------------------------------------------
def build_program(L=L_FULL, T=T_CHUNK, n_cores=N_CORES, repeat=1, **_ignored):
    C = L // T
    assert C * T == L
    ND = D_INNER // 128   # 8 d-blocks
    NM = D_MODEL // 128   # 4 k-tiles of d_model
    NF = D_FF // 128      # 16 m-tiles of d_ff

    nc = bass.Bass("TRN2", target_bir_lowering=False, debug=False,
                   num_devices=n_cores)

    def par(name, shape, out=False, dt=F32):
        return nc.declare_dram_parameter(name, list(shape), dt, isOutput=out)

    xT = par("xT", (D_MODEL, L), dt=BF16)
    outT = par("outT", (D_MODEL, L), out=True)
    W = {}
    for p in ("f", "b"):
        W[p] = dict(
            z_wT=par(f"{p}_z_wT", (D_MODEL, D_INNER), dt=BF16),
            cw0T=par(f"{p}_cw0T", (D_MODEL, D_INNER), dt=BF16),
            cw1T=par(f"{p}_cw1T", (D_MODEL, D_INNER), dt=BF16),
            cw2T=par(f"{p}_cw2T", (D_MODEL, D_INNER), dt=BF16),
            cw3T=par(f"{p}_cw3T", (D_MODEL, D_INNER), dt=BF16),
            out_wT=par(f"{p}_out_wT", (D_INNER, D_MODEL), dt=BF16),
            xp_wT=par(f"{p}_xp_wT", (D_INNER, DT_RANK + 2 * D_STATE), dt=BF16),
            dt_wT=par(f"{p}_dt_wT", (DT_RANK, D_INNER), dt=BF16),
            conv_bT=par(f"{p}_conv_bT", (2, D_INNER), dt=BF16),
            dt_bT=par(f"{p}_dt_bT", (2, D_INNER), dt=BF16),
            D=par(f"{p}_D", (D_INNER, 1)),
        )
    LN = {k: par(k, (D_MODEL, 1)) for k in
          ("lnf_g", "lnf_b", "lnb_g", "lnb_b", "lnff_g", "lnff_b")}
    w1T = par("w1T", (D_MODEL, D_FF), dt=BF16)
    b1 = par("b1", (D_FF, 1))
    w2T = par("w2T", (D_FF, D_MODEL), dt=BF16)
    b2 = par("b2", (D_MODEL, 1))
    selbc = par("selbc", (48, 16 * 128), dt=BF16)
    ident = par("ident", (128, 128), dt=BF16)

    of_d = nc.dram_tensor("of_d", [D_MODEL, L], BF16)
    ob_d = nc.dram_tensor("ob_d", [D_MODEL, L], BF16)

    with tile.TileContext(nc) as tc:
        with tc.tile_pool(name="const", bufs=1) as cpool:
            ones_col = cpool.tile([128, 1], BF16, tag="ones_col", name="ones_col")
            nc.vector.memset(ones_col[:], 1.0)
            ones_row = cpool.tile([1, 128], BF16, tag="ones_row", name="ones_row")
            nc.vector.memset(ones_row[:], 1.0)
            eps_t = cpool.tile([1, 1], F32, tag="eps_t", name="eps_t")
            nc.vector.memset(eps_t[:], EPS)
            ones_rT = cpool.tile([2, T], BF16, tag="ones_rT", name="ones_rT")
            nc.vector.memset(ones_rT[:], 1.0)
            # Row-broadcast selection matmuls (B from dbc rows 32..47 base 32,
            # C from csb base 0); pattern shipped from host.
            ones_bc = cpool.tile([48, 16 * 128], BF16, tag="ones_bc",
                                 name="ones_bc")
            nc.sync.dma_start(ones_bc[:], selbc[:])
            id128 = cpool.tile([128, 128], BF16, tag="id128", name="id128")
            nc.sync.dma_start(id128[:], ident[:])

            for _rep in range(repeat):
                _direction(nc, tc, W["f"], LN, xT, of_d, True, "f",
                           L, T, C, ND, NM, ones_col, ones_row, ones_bc,
                           id128, eps_t, ones_rT, None)
                ffn = dict(w1T=w1T, b1=b1, w2T=w2T, b2=b2, of_d=of_d,
                           outT=outT)
                _direction(nc, tc, W["b"], LN, xT, None, False, "b",
                           L, T, C, ND, NM, ones_col, ones_row, ones_bc,
                           id128, eps_t, ones_rT, ffn)

    return nc


def _load_weights(nc, wp, w, lng_name, lnb_name, LN, ND, NM):
    """DMA per-direction weights into persistent sbuf tiles (all bf16),
    ordered by first use (cw/zw feed the first in_proj; outw/LN come a
    chunk later)."""
    s = {}
    s["cw"] = []
    for j in range(D_CONV):
        tiles = [wp.tile([128, D_INNER], BF16, tag=f"cw{j}_{k}",
                         name=f"cw{j}_{k}") for k in range(NM)]
        for k in range(NM):
            nc.sync.dma_start(tiles[k][:],
                              w[f"cw{j}T"][128 * k:128 * (k + 1), :])
        s["cw"].append(tiles)
    s["cbT"] = wp.tile([2, D_INNER], BF16, tag="cbT", name="cbT")
    nc.sync.dma_start(s["cbT"][:], w["conv_bT"][:])
    s["dbT"] = wp.tile([2, D_INNER], BF16, tag="dbT", name="dbT")
    nc.sync.dma_start(s["dbT"][:], w["dt_bT"][:])
    s["zw"] = [wp.tile([128, D_INNER], BF16, tag=f"zw{k}", name=f"zw{k}")
               for k in range(NM)]
    for k in range(NM):
        nc.sync.dma_start(s["zw"][k][:], w["z_wT"][128 * k:128 * (k + 1), :])
    s["xpw"] = [wp.tile([128, DT_RANK + 2 * D_STATE], BF16, tag=f"xpw{k}",
                        name=f"xpw{k}") for k in range(ND)]
    for k in range(ND):
        nc.sync.dma_start(s["xpw"][k][:], w["xp_wT"][128 * k:128 * (k + 1), :])
    s["dtw"] = wp.tile([DT_RANK, D_INNER], BF16, tag="dtw", name="dtw")
    nc.sync.dma_start(s["dtw"][:], w["dt_wT"][:])
    s["Dp"] = [wp.tile([128, 1], F32, tag=f"Dp{d}", name=f"Dp{d}")
               for d in range(ND)]
    for d in range(ND):
        nc.sync.dma_start(s["Dp"][d][:], w["D"][128 * d:128 * (d + 1), :])
    s["outw"] = [wp.tile([128, D_MODEL], BF16, tag=f"outw{k}", name=f"outw{k}")
                 for k in range(ND)]
    for k in range(ND):
        nc.sync.dma_start(s["outw"][k][:], w["out_wT"][128 * k:128 * (k + 1), :])
    s["lng"] = [wp.tile([128, 1], F32, tag=f"lng{k}", name=f"lng{k}")
                for k in range(NM)]
    s["lnb"] = [wp.tile([128, 1], F32, tag=f"lnb{k}", name=f"lnb{k}")
                for k in range(NM)]
    for k in range(NM):
        nc.sync.dma_start(s["lng"][k][:], LN[lng_name][128 * k:128 * (k + 1), :])
        nc.sync.dma_start(s["lnb"][k][:], LN[lnb_name][128 * k:128 * (k + 1), :])
    return s


def _layernorm(nc, ln_in, lng, lnb, psS, psM, smtmp, lnout_pool, ones_col,
               ones_row, eps_t, T, NM, tag, out_dt=BF16):
    """LN over the channel dim (NM k-tiles of 128 partitions).
    ln_in: list of NM bf16 sbuf tiles [128, T]. Returns output tiles."""
    ps_sq = psS.tile([1, 2 * T], F32, tag="stat", name="stat")
    ps_s = ps_sq[:, 0:T]
    ps_q = ps_sq[:, T:2 * T]
    sq = [None] * NM
    for k in range(NM):
        sq[k] = smtmp.tile([128, T], BF16, tag="sq", name="sq", bufs=4)
        nc.scalar.square(sq[k][:], ln_in[k][:])
    for k in range(NM):
        nc.tensor.matmul(ps_s, ones_col[:], ln_in[k][:],
                         start=(k == 0), stop=(k == NM - 1))
    for k in range(NM):
        nc.tensor.matmul(ps_q, ones_col[:], sq[k][:],
                         start=(k == 0), stop=(k == NM - 1))
    mu = smtmp.tile([1, T], BF16, tag="mu", name="mu", bufs=1)
    nc.scalar.activation(mu[:], ps_s, AF.Copy, scale=1.0 / D_MODEL)
    m2 = smtmp.tile([1, T], F32, tag="m2", name="m2", bufs=1)
    nc.scalar.activation(m2[:], ps_q, AF.Copy, scale=1.0 / D_MODEL)
    mu2 = smtmp.tile([1, T], F32, tag="mu2", name="mu2", bufs=1)
    nc.scalar.square(mu2[:], mu[:])
    var = smtmp.tile([1, T], F32, tag="var", name="var", bufs=1)
    nc.vector.tensor_sub(var[:], m2[:], mu2[:])
    # rstd = exp(-0.5*ln(var+eps))  (stays in the Exp/Ln ACT table)
    lnv = smtmp.tile([1, T], F32, tag="lnv", name="lnv", bufs=1)
    nc.scalar.activation(lnv[:], var[:], AF.Ln, bias=eps_t[:])
    rstd = smtmp.tile([1, T], BF16, tag="rstd", name="rstd", bufs=1)
    nc.scalar.activation(rstd[:], lnv[:], AF.Exp, scale=-0.5)
    # broadcast mu/rstd to 128 partitions; ACT-copy to bf16 sbuf
    ps_mr = psM.tile([128, 2 * T], F32, tag="bcst", name="bcst")
    nc.tensor.matmul(ps_mr[:, 0:T], ones_row[:], mu[:], start=True, stop=True)
    nc.tensor.matmul(ps_mr[:, T:2 * T], ones_row[:], rstd[:], start=True,
                     stop=True)
    mrb = smtmp.tile([128, 2 * T], BF16, tag="mrb", name="mrb", bufs=1)
    nc.scalar.copy(mrb[:], ps_mr[:])
    mub = mrb[:, 0:T]
    rsb = mrb[:, T:2 * T]
    outs = []
    for k in range(NM):
        t1 = smtmp.tile([128, T], BF16, tag="lt1", name="lt1", bufs=2)
        nc.vector.tensor_sub(t1[:], ln_in[k][:], mub)
        t2 = smtmp.tile([128, T], BF16, tag="lt2", name="lt2", bufs=2)
        nc.vector.tensor_mul(t2[:], t1[:], rsb)
        o = lnout_pool.tile([128, T], out_dt, tag=tag)
        nc.vector.tensor_scalar(o[:], t2[:], lng[k][:], lnb[k][:],
                                op0=ALU.mult, op1=ALU.add)
        outs.append(o)
    return outs


def _direction(nc, tc, w, LN, xT, stage_d, fwd, p, L, T, C, ND, NM,
               ones_col, ones_row, ones_bc, id128, eps_t, ones_rT, ffn):
    """One mamba direction, software-pipelined:
    per chunk: [shared prep ci] -> [deferred out_proj+LN of ci-1 (+FFN chunk
    when fused)] -> [d-loop ci with one-d-block-deferred reduce tails].
    When ffn is not None (backward direction), the FFN+final-LN for time
    chunk j=C-ci is emitted right after outLN(ci-1), consuming the backward
    LN outputs straight from SBUF (no DRAM staging)."""
    from contextlib import ExitStack
    T1 = T + 1
    doff = 1 if fwd else 0          # data column offset in [.., T1]
    boff = 0 if fwd else T          # breaker column
    zoff = 3 if fwd else 0          # data window offset in haloed xk
    with ExitStack() as ctx:
        wp = ctx.enter_context(tc.tile_pool(name=f"w_{p}", bufs=1))
        sw = _load_weights(nc, wp, w, f"ln{p}_g", f"ln{p}_b", LN, ND, NM)
        if ffn is not None:
            fw = _load_ffn_weights(nc, wp, ffn, LN, NM, D_FF // 128)

        xk_pool = ctx.enter_context(tc.tile_pool(name=f"xk_{p}", bufs=8))
        tmp_pool = ctx.enter_context(tc.tile_pool(name=f"tmp_{p}", bufs=3))
        xc_pool = ctx.enter_context(tc.tile_pool(name=f"xc_{p}", bufs=5))
        zs_pool = ctx.enter_context(tc.tile_pool(name=f"zs_{p}", bufs=5))
        g_pool = ctx.enter_context(tc.tile_pool(name=f"g_{p}", bufs=2))
        dbc_pool = ctx.enter_context(tc.tile_pool(name=f"dbc_{p}", bufs=2))
        repB_pool = ctx.enter_context(tc.tile_pool(name=f"repB_{p}", bufs=1))
        repC_pool = ctx.enter_context(tc.tile_pool(name=f"repC_{p}", bufs=1))
        dA_pool = ctx.enter_context(tc.tile_pool(name=f"dA_{p}", bufs=3))
        b_pool = ctx.enter_context(tc.tile_pool(name=f"b_{p}", bufs=2))
        st_pool = ctx.enter_context(tc.tile_pool(name=f"st_{p}", bufs=2))
        ys_pool = ctx.enter_context(tc.tile_pool(name=f"ys_{p}", bufs=8))
        y_pool = ctx.enter_context(tc.tile_pool(name=f"y_{p}", bufs=2))
        ln_pool = ctx.enter_context(tc.tile_pool(name=f"ln_{p}", bufs=4))
        lo_pool = ctx.enter_context(tc.tile_pool(name=f"lo_{p}", bufs=3))
        if ffn is not None:
            fio_pool = ctx.enter_context(tc.tile_pool(name="ffn_io", bufs=4))
            fh_pool = ctx.enter_context(tc.tile_pool(name="ffn_h", bufs=4))
            fh1_pool = ctx.enter_context(tc.tile_pool(name="ffn_h1", bufs=16))
            fln_pool = ctx.enter_context(tc.tile_pool(name="ffn_ln", bufs=4))
            flo_pool = ctx.enter_context(tc.tile_pool(name="ffn_lo", bufs=3))

        psA = ctx.enter_context(tc.tile_pool(name=f"psA_{p}", bufs=2, space="PSUM"))
        psB = ctx.enter_context(tc.tile_pool(name=f"psB_{p}", bufs=2, space="PSUM"))
        psS = ctx.enter_context(tc.tile_pool(name=f"psS_{p}", bufs=2, space="PSUM"))
        psM = ctx.enter_context(tc.tile_pool(name=f"psM_{p}", bufs=2, space="PSUM"))

        state_prev = [None] * ND
        pend = None   # (t0, xk, ys_tiles) of previous chunk

        def emit_outln(pd):
            pt0, pxk, pys = pd
            ln_in = [None] * NM
            for m in range(NM):
                ps = psA.tile([128, T], F32, tag="mm", name="mm")
                for k in range(ND):
                    nc.tensor.matmul(ps[:],
                                     sw["outw"][k][:, 128 * m:128 * (m + 1)],
                                     pys[k][:], start=(k == 0), stop=False)
                nc.tensor.matmul(ps[:], id128[:], pxk[m][:, zoff:zoff + T],
                                 start=False, stop=True)
                li = ln_pool.tile([128, T], BF16, tag="lnin", name="lnin")
                nc.scalar.copy(li[:], ps[:])
                ln_in[m] = li
            outs = _layernorm(nc, ln_in, sw["lng"], sw["lnb"], psS, psM,
                              tmp_pool, lo_pool, ones_col, ones_row, eps_t,
                              T, NM, tag="lo")
            if stage_d is not None:
                for m in range(NM):
                    nc.sync.dma_start(
                        stage_d[128 * m:128 * (m + 1), pt0:pt0 + T],
                        outs[m][:])
            return pt0, outs

        def emit_ffn_a(ob_tiles, of_tiles):
            NF = D_FF // 128
            hk = [None] * NM
            for k in range(NM):
                a = of_tiles[k]
                ps = psA.tile([128, T], F32, tag="mm", name="mm")
                nc.tensor.matmul(ps[:], id128[:], a[:], start=True, stop=False)
                nc.tensor.matmul(ps[:], id128[:], ob_tiles[k][:],
                                 start=False, stop=True)
                h = fh_pool.tile([128, T], BF16, tag="h", name="h")
                nc.scalar.activation(h[:], ps[:], AF.Copy, scale=0.5)
                hk[k] = h
            h1 = [None] * NF
            for m in range(NF):
                ps = psA.tile([128, T], F32, tag="mm", name="mm")
                for k in range(NM):
                    nc.tensor.matmul(ps[:],
                                     fw["w1"][k][:, 128 * m:128 * (m + 1)],
                                     hk[k][:], start=(k == 0),
                                     stop=(k == NM - 1))
                t = fh1_pool.tile([128, T], BF16, tag="h1", name="h1")
                nc.scalar.activation(t[:], ps[:], AF.Gelu_apprx_tanh,
                                     bias=fw["b1"][m][:])
                h1[m] = t
            return hk, h1

        def emit_ffn_b(t0, mid):
            NF = D_FF // 128
            hk, h1 = mid
            ln_in = [None] * NM
            for m in range(NM):
                ps = psA.tile([128, T], F32, tag="mm", name="mm")
                for k in range(NF):
                    nc.tensor.matmul(ps[:],
                                     fw["w2"][k][:, 128 * m:128 * (m + 1)],
                                     h1[k][:], start=(k == 0), stop=False)
                nc.tensor.matmul(ps[:], id128[:], hk[m][:],
                                 start=False, stop=True)
                li = fln_pool.tile([128, T], BF16, tag="lnin", name="lnin")
                nc.scalar.activation(li[:], ps[:], AF.Identity,
                                     bias=fw["b2"][m][:])
                ln_in[m] = li
            outs = _layernorm(nc, ln_in, fw["lng"], fw["lnb"], psS, psM,
                              tmp_pool, flo_pool, ones_col, ones_row, eps_t,
                              T, NM, tag="lo", out_dt=F32)
            for m in range(NM):
                nc.sync.dma_start(
                    ffn["outT"][128 * m:128 * (m + 1), t0:t0 + T], outs[m][:])

        for ci in range(C):
            j = ci if fwd else (C - 1 - ci)      # time-chunk index
            t0 = j * T

            # ---- shared prep: haloed x loads ----
            xk = []
            for k in range(NM):
                t = xk_pool.tile([128, T + 3], BF16, tag="xk", name="xk")
                r0, r1 = 128 * k, 128 * (k + 1)
                if fwd:
                    if ci == 0:
                        nc.vector.memset(t[:, 0:3], 0.0)
                        nc.sync.dma_start(t[:, 3:3 + T], xT[r0:r1, 0:T])
                    else:
                        nc.sync.dma_start(t[:], xT[r0:r1, t0 - 3:t0 + T])
                else:
                    if ci == 0:
                        nc.sync.dma_start(t[:, 0:T], xT[r0:r1, t0:t0 + T])
                        nc.vector.memset(t[:, T:T + 3], 0.0)
                    else:
                        nc.sync.dma_start(t[:], xT[r0:r1, t0:t0 + T + 3])
                xk.append(t)

            # ---- shared prep: fused in_proj+conv -> xc, z -> zs ----
            # pairs of d-blocks share one [128,2T] psum + ONE silu ACT;
            # conv bias enters via a rank-1 matmul (convbT x ones_row)
            xc_tiles = [None] * ND
            zs_tiles = [None] * ND
            for mp in range(ND // 2):
                ps2 = psB.tile([128, 2, T], F32, tag="bc", name="bc")
                for half in range(2):
                    m = 2 * mp + half
                    for jj in range(D_CONV):
                        for k in range(NM):
                            nc.tensor.matmul(
                                ps2[:, half, :],
                                sw["cw"][jj][k][:, 128 * m:128 * (m + 1)],
                                xk[k][:, jj:jj + T],
                                start=(jj == 0 and k == 0), stop=False)
                    nc.tensor.matmul(ps2[:, half, :],
                                     sw["cbT"][:, 128 * m:128 * (m + 1)],
                                     ones_rT[:], start=False, stop=True)
                xc2 = xc_pool.tile([128, 2, T], BF16, tag="xc", name="xc")
                nc.scalar.activation(xc2[:], ps2[:], AF.Silu)
                xc_tiles[2 * mp] = xc2[:, 0, :]
                xc_tiles[2 * mp + 1] = xc2[:, 1, :]
            for mp in range(ND // 2):
                ps2 = psB.tile([128, 2, T], F32, tag="bc", name="bc")
                for half in range(2):
                    m = 2 * mp + half
                    for k in range(NM):
                        nc.tensor.matmul(ps2[:, half, :],
                                         sw["zw"][k][:, 128 * m:128 * (m + 1)],
                                         xk[k][:, zoff:zoff + T],
                                         start=(k == 0), stop=(k == NM - 1))
                zs2 = zs_pool.tile([128, 2, T], BF16, tag="zs", name="zs")
                nc.scalar.activation(zs2[:], ps2[:], AF.Silu)
                zs_tiles[2 * mp] = zs2[:, 0, :]
                zs_tiles[2 * mp + 1] = zs2[:, 1, :]

            # ---- shared prep: xproj -> dbc [48,T], csb [16,T] ----
            psd = psA.tile([DT_RANK + D_STATE, T], F32, tag="mm", name="mm")
            for k in range(ND):
                nc.tensor.matmul(psd[:], sw["xpw"][k][:, :DT_RANK + D_STATE],
                                 xc_tiles[k][:], start=(k == 0),
                                 stop=(k == ND - 1))
            dbc = dbc_pool.tile([DT_RANK + D_STATE, T], BF16, tag="dbc",
                                name="dbc")
            nc.scalar.copy(dbc[:], psd[:])
            psc = psA.tile([D_STATE, T], F32, tag="mm", name="mm")
            for k in range(ND):
                nc.tensor.matmul(psc[:], sw["xpw"][k][:, DT_RANK + D_STATE:],
                                 xc_tiles[k][:], start=(k == 0),
                                 stop=(k == ND - 1))
            csb = dbc_pool.tile([D_STATE, T], BF16, tag="csb", name="csb")
            nc.scalar.copy(csb[:], psc[:])

            # ---- shared prep: broadcast B,C rows (2-state batches) ----
            Brep = repB_pool.tile([128, D_STATE, T], BF16, tag="brep",
                                 name="brep")
            Crep = repC_pool.tile([128, D_STATE, T], BF16, tag="crep",
                                 name="crep")
            for q in range(8):
                pb = psB.tile([128, 2, T], F32, tag="bc", name="bc")
                for n2 in range(2):
                    n = 2 * q + n2
                    nc.tensor.matmul(pb[:, n2, :],
                                     ones_bc[32:48, 128 * n:128 * (n + 1)],
                                     dbc[DT_RANK:DT_RANK + D_STATE, :],
                                     start=True, stop=True)
                nc.scalar.copy(Brep[:, 2 * q:2 * (q + 1), :], pb[:])
                pc = psB.tile([128, 2, T], F32, tag="bc", name="bc")
                for n2 in range(2):
                    n = 2 * q + n2
                    nc.tensor.matmul(pc[:, n2, :],
                                     ones_bc[0:16, 128 * n:128 * (n + 1)],
                                     csb[:], start=True, stop=True)
                nc.scalar.copy(Crep[:, 2 * q:2 * (q + 1), :], pc[:])

            # ---- prefetch FFN of-tiles for the NEXT slot's time chunk ----
            of_next = None
            if ffn is not None and ci >= 1:
                jf = C - ci            # ffn time-chunk consumed in this slot
                of_next = []
                for k in range(NM):
                    a = fio_pool.tile([128, T], BF16, tag="of", name="of")
                    nc.sync.dma_start(
                        a[:], ffn["of_d"][128 * k:128 * (k + 1),
                                          jf * T:jf * T + T])
                    of_next.append(a)

            # ---- d-loop; deferred out_proj+LN(+FFN) spread over d slots ----
            ys_tiles = [None] * ND
            tails = []
            pt0 = bouts = None
            dt_pair = [None] * (ND // 2)
            for d in range(ND):
                if d == 2 and pend is not None:
                    pt0, bouts = emit_outln(pend)
                if d == 3 and bouts is not None and ffn is not None:
                    ffn_mid = emit_ffn_a(bouts, of_tiles)
                if d == 6 and bouts is not None and ffn is not None:
                    emit_ffn_b(pt0, ffn_mid)
                of_tiles = of_next
                if d % 2 == 0:
                    # paired dt = softplus(u+dt_b) = ln(exp(u+dt_b)+1)
                    dp = d // 2
                    ps2 = psB.tile([128, 2, T], F32, tag="bc", name="bc")
                    for half in range(2):
                        dd = d + half
                        nc.tensor.matmul(
                            ps2[:, half, :],
                            sw["dtw"][:, 128 * dd:128 * (dd + 1)],
                            dbc[0:DT_RANK, :], start=True, stop=False)
                        nc.tensor.matmul(ps2[:, half, :],
                                         sw["dbT"][:, 128 * dd:128 * (dd + 1)],
                                         ones_rT[:], start=False, stop=True)
                    e2 = tmp_pool.tile([128, 2, T], BF16, tag="e", name="e",
                                       bufs=2)
                    nc.scalar.activation(e2[:], ps2[:], AF.Exp)
                    dt2 = tmp_pool.tile([128, 2, T], BF16, tag="dt",
                                        name="dt", bufs=2)
                    nc.scalar.activation(dt2[:], e2[:], AF.Ln, bias=1.0)
                    dt_pair[dp] = dt2
                dtt = dt_pair[d // 2][:, d % 2, :]
                g_t = g_pool.tile([128, T], BF16, tag="g", name="g")
                nc.vector.tensor_mul(g_t[:], dtt, xc_tiles[d][:])

                dA = dA_pool.tile([128, D_STATE, T1], BF16, tag="dA",
                                  name="dA")

                def dpl(i):
                    return dA[:, i, doff:doff + T]

                # powers w^1..w^16 in 1 ACT + 4 doubling DVE muls:
                # p1=w*w; [p2,p3]=[p0,p1]*p1; [p4..7]=[p0..3]*p3;
                # [p8..15]=[p0..7]*p7
                nc.scalar.activation(dpl(0), dtt, AF.Exp, scale=-1.0)
                nc.vector.tensor_mul(dpl(1), dpl(0), dpl(0))
                b2 = dA[:, 1, doff:doff + T].unsqueeze(1).broadcast_to(
                    [128, 2, T])
                nc.vector.tensor_mul(dA[:, 2:4, doff:doff + T], b2,
                                     dA[:, 0:2, doff:doff + T])
                b4 = dA[:, 3, doff:doff + T].unsqueeze(1).broadcast_to(
                    [128, 4, T])
                nc.vector.tensor_mul(dA[:, 4:8, doff:doff + T], b4,
                                     dA[:, 0:4, doff:doff + T])
                b8 = dA[:, 7, doff:doff + T].unsqueeze(1).broadcast_to(
                    [128, 8, T])
                nc.vector.tensor_mul(dA[:, 8:16, doff:doff + T], b8,
                                     dA[:, 0:8, doff:doff + T])
                nc.vector.memset(dA[:, :, boff:boff + 1], 0.0)

                # b = g * B; breaker col = carried state
                bt = b_pool.tile([128, D_STATE, T1], BF16, tag="b", name="b")
                gb = g_t[:].unsqueeze(1).broadcast_to([128, D_STATE, T])
                nc.vector.tensor_mul(bt[:, :, doff:doff + T], gb,
                                     Brep[:, :, :])
                if ci == 0:
                    nc.vector.memset(bt[:, :, boff:boff + 1], 0.0)
                else:
                    nc.vector.tensor_copy(bt[:, :, boff:boff + 1],
                                          state_prev[d][:].unsqueeze(2))
                flat_a = dA[:, :, :].rearrange("p n t -> p (n t)")
                flat_b = bt[:, :, :].rearrange("p n t -> p (n t)")
                if fwd:
                    nc.vector.tensor_tensor_scan(
                        flat_b, flat_a, flat_b, 0.0,
                        op0=ALU.mult, op1=ALU.add)
                else:
                    nc.vector.tensor_tensor_scan(
                        flat_b[:, ::-1], flat_a[:, ::-1], flat_b[:, ::-1],
                        0.0, op0=ALU.mult, op1=ALU.add)
                stt = st_pool.tile([128, D_STATE], BF16, tag=f"st{d}",
                                   name=f"st{d}")
                nc.vector.tensor_copy(stt[:], bt[:, :, T if fwd else 0])
                state_prev[d] = stt
                # yterm = h*C written into the dead dA buffer; gpsimd does
                # the top yt half + two fold levels (16->8->4), DVE the rest;
                # the DVE tail is deferred TWO d-blocks so Pool never stalls it
                yt = dA[:, :, doff:doff + T]
                nc.vector.tensor_mul(yt[:, 0:8, :],
                                     bt[:, 0:8, doff:doff + T],
                                     Crep[:, 0:8, :])
                nc.gpsimd.tensor_mul(yt[:, 8:16, :],
                                     bt[:, 8:16, doff:doff + T],
                                     Crep[:, 8:16, :])
                nc.gpsimd.tensor_add(yt[:, 0:8, :], yt[:, 0:8, :],
                                     yt[:, 8:16, :])
                nc.gpsimd.tensor_add(yt[:, 0:4, :], yt[:, 0:4, :],
                                     yt[:, 4:8, :])

                if len(tails) >= 2:
                    tails.pop(0)()

                def make_tail(d=d, t8=yt):
                    def run():
                        nc.vector.tensor_add(t8[:, 0:2, :], t8[:, 0:2, :],
                                             t8[:, 2:4, :])
                        y_t = y_pool.tile([128, T], BF16, tag="y", name="y")
                        nc.vector.tensor_add(y_t[:], t8[:, 0, :], t8[:, 1, :])
                        yg = y_pool.tile([128, T], BF16, tag="yg", name="yg")
                        nc.vector.scalar_tensor_tensor(
                            yg[:], xc_tiles[d][:], sw["Dp"][d][:], y_t[:],
                            op0=ALU.mult, op1=ALU.add)
                        ys = ys_pool.tile([128, T], BF16, tag="ys", name="ys")
                        nc.vector.tensor_mul(ys[:], yg[:], zs_tiles[d][:])
                        ys_tiles[d] = ys
                    return run

                tails.append(make_tail())
            for tl in tails:
                tl()
            tails = []
            pend = (t0, xk, ys_tiles)

        if ffn is not None:
            of_last = []
            for k in range(NM):
                a = fio_pool.tile([128, T], BF16, tag="of", name="of")
                nc.sync.dma_start(a[:], ffn["of_d"][128 * k:128 * (k + 1),
                                                    0:T])
                of_last.append(a)
        pt0, bouts = emit_outln(pend)
        if ffn is not None:
            mid = emit_ffn_a(bouts, of_last)
            emit_ffn_b(pt0, mid)


def _load_ffn_weights(nc, wp, ffn, LN, NM, NF):
    fw = {}
    fw["w1"] = [wp.tile([128, D_FF], BF16, tag=f"w1_{k}", name=f"w1_{k}")
                for k in range(NM)]
    for k in range(NM):
        nc.sync.dma_start(fw["w1"][k][:], ffn["w1T"][128 * k:128 * (k + 1), :])
    fw["w2"] = [wp.tile([128, D_MODEL], BF16, tag=f"w2_{k}", name=f"w2_{k}")
                for k in range(NF)]
    for k in range(NF):
        nc.sync.dma_start(fw["w2"][k][:], ffn["w2T"][128 * k:128 * (k + 1), :])
    fw["b1"] = [wp.tile([128, 1], F32, tag=f"fb1_{m}", name=f"fb1_{m}")
                for m in range(NF)]
    for m in range(NF):
        nc.sync.dma_start(fw["b1"][m][:], ffn["b1"][128 * m:128 * (m + 1), :])
    fw["b2"] = [wp.tile([128, 1], F32, tag=f"fb2_{m}", name=f"fb2_{m}")
                for m in range(NM)]
    for m in range(NM):
        nc.sync.dma_start(fw["b2"][m][:], ffn["b2"][128 * m:128 * (m + 1), :])
    fw["lng"] = [wp.tile([128, 1], F32, tag=f"flng{k}", name=f"flng{k}")
                 for k in range(NM)]
    fw["lnb"] = [wp.tile([128, 1], F32, tag=f"flnb{k}", name=f"flnb{k}")
                 for k in range(NM)]
    for k in range(NM):
        nc.sync.dma_start(fw["lng"][k][:],
                          LN["lnff_g"][128 * k:128 * (k + 1), :])
        nc.sync.dma_start(fw["lnb"][k][:],
                          LN["lnff_b"][128 * k:128 * (k + 1), :])
    return fw


# ----------------------------------------------------------------------------
# host side: input packing, cached jitted runner
# ----------------------------------------------------------------------------
def pack_inputs(inputs, n_cores=N_CORES):
    """Shared weight map + per-core input maps. Host-side layout prep:
    transposes, bf16 casts, conv folded into 4 shifted in_proj matrices."""
    f32 = np.float32
    try:
        import ml_dtypes
        bf16 = ml_dtypes.bfloat16
    except ImportError:
        import jax.numpy as jnp
        bf16 = jnp.bfloat16

    def t(a, dt=None):
        return np.ascontiguousarray(
            np.asarray(a, f32).T.astype(dt if dt else f32))

    shared = {}
    for p in ("f", "b"):
        in_w = np.asarray(inputs[f"{p}_in_w"], f32)       # (2048, 512)
        conv_w = np.asarray(inputs[f"{p}_conv_w"], f32)   # (1024, 4)
        xi_w = in_w[:D_INNER]                             # (1024, 512)
        z_w = in_w[D_INNER:]
        shared[f"{p}_z_wT"] = t(z_w, bf16)
        for o in range(D_CONV):
            # device tap offset o reads x[t0-3+o] (fwd) / x[t0+o] i.e.
            # x[t+(3-s)] (bwd); fwd: weight s=o; bwd: weight s=3-o.
            s = o if p == "f" else 3 - o
            Wj = xi_w * conv_w[:, s:s + 1]                # (1024, 512)
            shared[f"{p}_cw{o}T"] = t(Wj, bf16)
        shared[f"{p}_out_wT"] = t(inputs[f"{p}_out_w"], bf16)
        shared[f"{p}_xp_wT"] = t(inputs[f"{p}_xproj_w"], bf16)
        shared[f"{p}_dt_wT"] = t(inputs[f"{p}_dt_w"], bf16)
        def _hilo(v):
            v = np.asarray(v, f32).reshape(1, -1)
            hi = v.astype(bf16)
            lo = (v - hi.astype(f32)).astype(bf16)
            return np.concatenate([hi, lo], axis=0)

        shared[f"{p}_conv_bT"] = _hilo(inputs[f"{p}_conv_b"])
        shared[f"{p}_dt_bT"] = _hilo(inputs[f"{p}_dt_b"])
        shared[f"{p}_D"] = np.asarray(inputs[f"{p}_D"], f32).reshape(-1, 1)
    for src, dst in (("ln_f_g", "lnf_g"), ("ln_f_b", "lnf_b"),
                     ("ln_b_g", "lnb_g"), ("ln_b_b", "lnb_b"),
                     ("ln_ff_g", "lnff_g"), ("ln_ff_b", "lnff_b")):
        shared[dst] = np.asarray(inputs[src], f32).reshape(-1, 1)
    shared["w1T"] = t(inputs["ffn_w1"], bf16)
    shared["b1"] = np.asarray(inputs["ffn_b1"], f32).reshape(-1, 1)
    shared["w2T"] = t(inputs["ffn_w2"], bf16)
    shared["b2"] = np.asarray(inputs["ffn_b2"], f32).reshape(-1, 1)
    sel = np.zeros((48, 16 * 128), f32)
    for k in range(D_STATE):
        sel[k, 128 * k:128 * (k + 1)] = 1.0
        sel[32 + k, 128 * k:128 * (k + 1)] = 1.0
    shared["selbc"] = sel.astype(bf16)
    shared["ident"] = np.eye(128, dtype=f32).astype(bf16)

    x = np.asarray(inputs["x"], f32)
    in_maps = []
    for i in range(n_cores):
        m = dict(shared)
        m["xT"] = np.ascontiguousarray(x[i].T.astype(bf16))
        in_maps.append(m)
    return in_maps


_RUNNER = {}


def make_runner(**build_kwargs):
    import jax
    import jax.numpy as jnp
    from jax.experimental.shard_map import shard_map
    from jax.sharding import Mesh, NamedSharding, PartitionSpec
    from concourse import bass2jax

    nc = build_program(**build_kwargs)
    split_multi_waits(nc)
    bass2jax.install_neuronx_cc_hook()

    partition_name = (nc.partition_id_tensor.name
                      if nc.partition_id_tensor else None)
    in_names, out_names, out_avals, zero_shapes = [], [], [], []
    for alloc in nc.m.functions[0].allocations:
        if not isinstance(alloc, mybir.MemoryLocationSet):
            continue
        name = alloc.memorylocations[0].name
        if alloc.kind == "ExternalInput":
            if name != partition_name:
                in_names.append(name)
        elif alloc.kind == "ExternalOutput":
            shape = tuple(alloc.tensor_shape)
            dtype = mybir.dt.np(alloc.dtype)
            out_names.append(name)
            out_avals.append(jax.core.ShapedArray(shape, dtype))
            zero_shapes.append((shape, dtype))
    n_params = len(in_names)
    all_in_names = list(in_names) + list(out_names)
    if partition_name is not None:
        all_in_names.append(partition_name)

    def _body(*args):
        operands = list(args)
        if partition_name is not None:
            operands.append(bass2jax.partition_id_tensor())
        outs = bass2jax._bass_exec_p.bind(
            *operands,
            out_avals=tuple(out_avals),
            in_names=tuple(all_in_names),
            out_names=tuple(out_names),
            lowering_input_output_aliases=(),
            sim_require_finite=True,
            sim_require_nnan=True,
            nc=nc,
        )
        return tuple(outs)

    devices = jax.devices()[:N_CORES]
    mesh = Mesh(np.asarray(devices), ("core",))
    n_outs = len(out_avals)
    in_specs = (PartitionSpec("core"),) * (n_params + n_outs)
    out_specs = (PartitionSpec("core"),) * n_outs
    donate = tuple(range(n_params, n_params + n_outs))
    sharded = jax.jit(
        shard_map(_body, mesh=mesh, in_specs=in_specs, out_specs=out_specs,
                  check_rep=False),
        donate_argnums=donate, keep_unused=True)

    sh = NamedSharding(mesh, PartitionSpec("core"))

    def make_zeros():
        return tuple(
            jnp.zeros((N_CORES * s[0],) + tuple(s[1:]), d)
            for s, d in zero_shapes)

    zeros_fn = jax.jit(make_zeros, out_shardings=(sh,) * n_outs)

    return dict(
        fn=sharded, in_names=in_names, out_names=out_names,
        out_avals=out_avals, zeros_fn=zeros_fn, mesh=mesh, sh=sh, jnp=jnp,
        jax=jax)


BEST_CONFIG = dict()


def _get_runner():
    if not _RUNNER:
        _RUNNER.update(make_runner(**BEST_CONFIG))
    return _RUNNER


def _device_inputs(in_maps, r=None):
    import jax
    r = r or _get_runner()
    concat = [np.concatenate([in_maps[c][n] for c in range(N_CORES)], axis=0)
              for n in r["in_names"]]
    return [jax.device_put(a, r["sh"]) for a in concat]


def _run_once(dev_in, r=None):
    r = r or _get_runner()
    zeros = r["zeros_fn"]()
    outs = r["fn"](*dev_in, *zeros)
    return outs


def kernel(**inputs):
    r = _get_runner()
    in_maps = pack_inputs(inputs)
    dev_in = _device_inputs(in_maps)
    outs = _run_once(dev_in)
    outT = np.asarray(outs[r["out_names"].index("outT")])
    outT = outT.reshape(N_CORES, D_MODEL, L_FULL)
    out = np.ascontiguousarray(np.transpose(outT, (0, 2, 1)).astype(np.float32))
    return out
